# revision 3
# baseline (speedup 1.0000x reference)
"""GQA kernel for trn2, 8 cores: DP over batch (2) x TP over kv-head groups (4).

Each core computes, for its (batch b, kv-group g):
  - qkv projection for its 4 q-heads + 1 kv-head (q pre-scaled by 1/sqrt(dk))
  - RoPE on q/k
  - full (non-causal) attention for the 4 q-heads vs its kv-head
  - partial out-projection with its 2048 rows of W_out
Host sums the 4 per-group partials per batch and adds bias.

Self-contained: hardcodes all shapes. kernel(**inputs) -> np.ndarray.
"""

import math
from contextlib import ExitStack

import numpy as np

import concourse.bass as bass
import concourse.bacc as bacc
import concourse.tile as tile
import concourse.mybir as mybir
from concourse.bass_utils import run_bass_kernel_spmd
from concourse.masks import make_identity

F32 = mybir.dt.float32
L = 2048          # sequence length
D = 2048          # d_model
DK = 128          # head dim (q/k)
DV = 512          # head dim (v)
NHQ = 4           # q heads per core
CQK = NHQ * DK + DK   # 640 qk projection cols per core
NI = 4            # query chunks of 512
NJ = 16           # key chunks of 128
NDCH = 16         # d_model chunks of 128

_NC_CACHE = {}


def build_nc():
    if "nc" in _NC_CACHE:
        return _NC_CACHE["nc"]
    nc = bacc.Bacc("TRN2", target_bir_lowering=False, debug=False)

    x_d = nc.dram_tensor("x", [L, D], F32, kind="ExternalInput")
    wqk_d = nc.dram_tensor("wqk", [D, CQK], F32, kind="ExternalInput")
    wv_d = nc.dram_tensor("wv", [D, DV], F32, kind="ExternalInput")
    wo_d = nc.dram_tensor("wo", [NHQ * DV, D], F32, kind="ExternalInput")
    cos_d = nc.dram_tensor("cost", [DK, L], F32, kind="ExternalInput")
    sin_d = nc.dram_tensor("sint", [DK, L], F32, kind="ExternalInput")
    out_d = nc.dram_tensor("out", [L, D], F32, kind="ExternalOutput")

    EXP = mybir.ActivationFunctionType.Exp

    with ExitStack() as ctx:
        tc = ctx.enter_context(tile.TileContext(nc))
        # pools
        persist = ctx.enter_context(tc.tile_pool(name="persist", bufs=1))
        psS = ctx.enter_context(tc.tile_pool(name="psS", bufs=6, space="PSUM"))
        psA = ctx.enter_context(tc.tile_pool(name="psA", bufs=2, space="PSUM"))

        ident = persist.tile([128, 128], F32)
        make_identity(nc, ident)
        ones = persist.tile([128, 1], F32)
        nc.vector.memset(ones, 1.0)

        qT = persist.tile([128, NHQ, L], F32)      # [dk, h, pos]
        kT = persist.tile([128, L], F32)           # [dk, pos]
        v_sb = persist.tile([128, NJ, DV], F32)    # [key_in_chunk, key_chunk, e]

        # ---------------- Phase B: x^T, qkv projection, rope ----------------
        with tc.tile_pool(name="pb1", bufs=1) as pb1, \
             tc.tile_pool(name="pb2", bufs=2) as pb2:
            cosT = pb1.tile([128, L], F32)
            sinT = pb1.tile([128, L], F32)
            nc.sync.dma_start(out=cosT, in_=cos_d.ap())
            nc.sync.dma_start(out=sinT, in_=sin_d.ap())
            wv_sb = pb1.tile([128, NDCH, DV], F32)
            nc.sync.dma_start(
                out=wv_sb, in_=wv_d.ap().rearrange("(t p) c -> p t c", p=128))

            for i in range(NI):
                xT = pb1.tile([128, NDCH, 512], F32, tag="xT")
                # transpose x rows for this 512-query chunk
                for lsub in range(4):
                    l0 = i * 512 + lsub * 128
                    xn = pb2.tile([128, D], F32, tag="xn")
                    nc.sync.dma_start(out=xn, in_=x_d.ap()[l0:l0 + 128, :])
                    for dgrp in range(4):
                        ps = psS.tile([128, 512], F32, tag="stream")
                        for k in range(4):
                            dch = dgrp * 4 + k
                            nc.tensor.transpose(
                                ps[:, k * 128:(k + 1) * 128],
                                xn[:, dch * 128:(dch + 1) * 128], ident)
                        nc.vector.tensor_copy(
                            out=xT[:, dgrp * 4:dgrp * 4 + 4,
                                   lsub * 128:(lsub + 1) * 128],
                            in_=ps.rearrange("p (a b) -> p a b", a=4))

                # q/k projection + rope (c = 0..3 q heads, c = 4 is k)
                for c in range(5):
                    wq_t = pb2.tile([128, NDCH, 128], F32, tag="wqk")
                    nc.sync.dma_start(
                        out=wq_t,
                        in_=wqk_d.ap()[:, c * 128:(c + 1) * 128]
                            .rearrange("(t p) c -> p t c", p=128))
                    ps = psA.tile([128, 512], F32, tag="acc")
                    for t in range(NDCH):
                        nc.tensor.matmul(ps, lhsT=wq_t[:, t, :], rhs=xT[:, t, :],
                                         start=(t == 0), stop=(t == NDCH - 1))
                    isl = slice(i * 512, (i + 1) * 512)
                    dest = qT[:, c, isl] if c < NHQ else kT[:, isl]
                    cs = cosT[:, isl]
                    sn = sinT[:, isl]
                    tmp = pb2.tile([128, 512], F32, tag="rope")
                    nc.vector.tensor_mul(tmp[0:64, :], ps[64:128, :], sn[0:64, :])
                    nc.vector.tensor_mul(tmp[64:128, :], ps[0:64, :], sn[64:128, :])
                    nc.vector.tensor_mul(dest, ps, cs)
                    nc.vector.tensor_sub(dest[0:64, :], dest[0:64, :], tmp[0:64, :])
                    nc.vector.tensor_add(dest[64:128, :], dest[64:128, :],
                                         tmp[64:128, :])

                # v projection for these 4 key chunks
                for lsub in range(4):
                    ps = psA.tile([128, 512], F32, tag="acc")
                    for t in range(NDCH):
                        nc.tensor.matmul(
                            ps, lhsT=xT[:, t, lsub * 128:(lsub + 1) * 128],
                            rhs=wv_sb[:, t, :],
                            start=(t == 0), stop=(t == NDCH - 1))
                    nc.scalar.copy(out=v_sb[:, i * 4 + lsub, :], in_=ps)

        # ---------------- Phase C+D: attention + fused out-projection -------
        with tc.tile_pool(name="pc1", bufs=1) as pc1, \
             tc.tile_pool(name="pc2", bufs=2) as pc2:
            for i in range(NI):
                isl = slice(i * 512, (i + 1) * 512)
                ctxTs = []
                for h in range(NHQ):
                    expS = pc1.tile([128, NJ, 512], F32, tag="expS")
                    for j in range(NJ):
                        ps = psS.tile([128, 512], F32, tag="stream")
                        nc.tensor.matmul(ps, lhsT=kT[:, j * 128:(j + 1) * 128],
                                         rhs=qT[:, h, isl])
                        nc.scalar.activation(out=expS[:, j, :], in_=ps, func=EXP)
                    # row sums via ones-matmul (sum over keys = partition dim)
                    pso = psA.tile([1, 512], F32, tag="acc")
                    for j in range(NJ):
                        nc.tensor.matmul(pso, lhsT=ones[:, 0:1], rhs=expS[:, j, :],
                                         start=(j == 0), stop=(j == NJ - 1))
                    recip = pc1.tile([1, 512], F32, tag="recip")
                    nc.vector.reciprocal(recip, pso)
                    rb = pc1.tile([128, 512], F32, tag="rb")
                    nc.gpsimd.partition_broadcast(rb, recip)
                    ctxT = pc1.tile([128, 4, 512], F32, tag=f"ctx{h}")
                    for ec in range(4):
                        ps = psA.tile([128, 512], F32, tag="acc")
                        for j in range(NJ):
                            nc.tensor.matmul(
                                ps, lhsT=v_sb[:, j, ec * 128:(ec + 1) * 128],
                                rhs=expS[:, j, :],
                                start=(j == 0), stop=(j == NJ - 1))
                        nc.vector.tensor_mul(ctxT[:, ec, :], ps, rb)
                    ctxTs.append(ctxT)

                # fused out-projection for this query chunk
                for dm in range(4):
                    wo_t = pc2.tile([128, NDCH, 512], F32, tag="wo")
                    nc.sync.dma_start(
                        out=wo_t,
                        in_=wo_d.ap()[:, dm * 512:(dm + 1) * 512]
                            .rearrange("(t p) c -> p t c", p=128))
                    for lsub in range(4):
                        ps = psA.tile([128, 512], F32, tag="acc")
                        for h in range(NHQ):
                            for ec in range(4):
                                t = h * 4 + ec
                                nc.tensor.matmul(
                                    ps,
                                    lhsT=ctxTs[h][:, ec,
                                                  lsub * 128:(lsub + 1) * 128],
                                    rhs=wo_t[:, t, :],
                                    start=(t == 0), stop=(t == 15))
                        ost = pc1.tile([128, 512], F32, tag="ost")
                        nc.scalar.copy(out=ost, in_=ps)
                        l0 = i * 512 + lsub * 128
                        nc.sync.dma_start(
                            out=out_d.ap()[l0:l0 + 128, dm * 512:(dm + 1) * 512],
                            in_=ost)

    nc.compile()
    _NC_CACHE["nc"] = nc
    return nc


def make_core_inputs(x, W_attn, W_out):
    """Split full inputs into 8 per-core input maps (core = b*4 + g)."""
    Q_DIM = 2048
    K_DIM = 512
    scale = np.float32(1.0 / math.sqrt(DK))

    # rope tables, mirroring the fp32 reference computation
    inv_freq = (np.float32(1.0) /
                (np.float32(10000.0) **
                 (np.arange(0, DK, 2, dtype=np.float32) / np.float32(DK))))
    freqs = np.arange(L, dtype=np.float32)[:, None] * inv_freq[None, :]  # [L,64]
    ang = np.concatenate([freqs, freqs], axis=-1)  # [L, 128]
    cosT = np.ascontiguousarray(np.cos(ang).T.astype(np.float32))  # [128, L]
    sinT = np.ascontiguousarray(np.sin(ang).T.astype(np.float32))

    in_maps = []
    for core in range(8):
        b, g = divmod(core, 4)
        wq = (W_attn[:, 512 * g:512 * (g + 1)] * scale).astype(np.float32)
        wk = W_attn[:, Q_DIM + 128 * g:Q_DIM + 128 * (g + 1)]
        wqk = np.ascontiguousarray(np.concatenate([wq, wk], axis=1))
        wv = np.ascontiguousarray(W_attn[:, Q_DIM + K_DIM + 512 * g:
                                         Q_DIM + K_DIM + 512 * (g + 1)])
        wo = np.ascontiguousarray(W_out[2048 * g:2048 * (g + 1), :])
        in_maps.append({
            "x": np.ascontiguousarray(x[b]).astype(np.float32),
            "wqk": wqk.astype(np.float32),
            "wv": wv.astype(np.float32),
            "wo": wo.astype(np.float32),
            "cost": cosT,
            "sint": sinT,
        })
    return in_maps


def kernel(x, W_attn, W_out, b_out, _trace=False, _trace_cores=None):
    x = np.asarray(x)
    W_attn = np.asarray(W_attn)
    W_out = np.asarray(W_out)
    b_out = np.asarray(b_out)
    nc = build_nc()
    in_maps = make_core_inputs(x, W_attn, W_out)
    res = run_bass_kernel_spmd(
        nc, in_maps, core_ids=list(range(8)),
        trace=_trace, trace_cores=_trace_cores)
    parts = [res.results[c]["out"] for c in range(8)]
    out = np.empty((2, L, D), dtype=np.float32)
    for b in range(2):
        acc = parts[4 * b].astype(np.float32)
        for g in range(1, 4):
            acc = acc + parts[4 * b + g]
        out[b] = acc + b_out[None, :].astype(np.float32)
    if _trace:
        kernel._last_results = res
    return out


# revision 6
# speedup vs baseline: 3.5285x; 3.5285x over previous
"""GQA kernel for trn2, 8 cores: DP over batch (2) x TP over kv-head groups (4).

Each core computes, for its (batch b, kv-group g):
  - qkv projection for its 4 q-heads + 1 kv-head (q pre-scaled by 1/sqrt(dk))
  - RoPE on q/k
  - full (non-causal) attention for the 4 q-heads vs its kv-head
  - partial out-projection with its 2048 rows of W_out
Host sums the 4 per-group partials per batch and adds bias.

Matmul operands are bf16 (PE runs fp32 at 1/4 rate; bf16 is full rate).
Accumulation and softmax statistics stay fp32.

Self-contained: hardcodes all shapes. kernel(**inputs) -> np.ndarray.
"""

import math
from contextlib import ExitStack

import numpy as np
import ml_dtypes

import concourse.bass as bass
import concourse.bacc as bacc
import concourse.tile as tile
import concourse.mybir as mybir
from concourse.bass_utils import run_bass_kernel_spmd
from concourse.masks import make_identity

F32 = mybir.dt.float32
BF16 = mybir.dt.bfloat16
L = 2048          # sequence length
D = 2048          # d_model
DK = 128          # head dim (q/k)
DV = 512          # head dim (v)
NHQ = 4           # q heads per core
CQK = NHQ * DK + DK   # 640 qk projection cols per core
NI = 4            # query chunks of 512
NJ = 16           # key chunks of 128
NDCH = 16         # d_model chunks of 128

_NC_CACHE = {}


def build_nc():
    if "nc" in _NC_CACHE:
        return _NC_CACHE["nc"]
    nc = bacc.Bacc("TRN2", target_bir_lowering=False, debug=False)

    x_d = nc.dram_tensor("x", [L, D], BF16, kind="ExternalInput")
    wqk_d = nc.dram_tensor("wqk", [D, CQK], BF16, kind="ExternalInput")
    wv_d = nc.dram_tensor("wv", [D, DV], BF16, kind="ExternalInput")
    wo_d = nc.dram_tensor("wo", [NHQ * DV, D], BF16, kind="ExternalInput")
    cos_d = nc.dram_tensor("cost", [DK, L], F32, kind="ExternalInput")
    sin_d = nc.dram_tensor("sint", [DK, L], F32, kind="ExternalInput")
    out_d = nc.dram_tensor("out", [L, D], F32, kind="ExternalOutput")

    EXP = mybir.ActivationFunctionType.Exp

    with ExitStack() as ctx:
        tc = ctx.enter_context(tile.TileContext(nc))
        # pools
        persist = ctx.enter_context(tc.tile_pool(name="persist", bufs=1))
        psS = ctx.enter_context(tc.tile_pool(name="psS", bufs=6, space="PSUM"))
        psA = ctx.enter_context(tc.tile_pool(name="psA", bufs=2, space="PSUM"))

        ident = persist.tile([128, 128], BF16)
        make_identity(nc, ident)
        ones = persist.tile([128, 1], BF16)
        nc.vector.memset(ones, 1.0)

        qT = persist.tile([128, NHQ, L], BF16)      # [dk, h, pos]
        kT = persist.tile([128, L], BF16)           # [dk, pos]
        v_sb = persist.tile([128, NJ, DV], BF16)    # [key_in_chunk, key_chunk, e]

        # ---------------- Phase B: x^T, qkv projection, rope ----------------
        with tc.tile_pool(name="pb1", bufs=1) as pb1, \
             tc.tile_pool(name="pb2", bufs=2) as pb2:
            cosT = pb1.tile([128, L], F32)
            sinT = pb1.tile([128, L], F32)
            nc.sync.dma_start(out=cosT, in_=cos_d.ap())
            nc.sync.dma_start(out=sinT, in_=sin_d.ap())
            wv_sb = pb1.tile([128, NDCH, DV], BF16)
            nc.sync.dma_start(
                out=wv_sb, in_=wv_d.ap().rearrange("(t p) c -> p t c", p=128))

            for i in range(NI):
                xT = pb1.tile([128, NDCH, 512], BF16, tag="xT")
                # transpose x rows for this 512-query chunk
                for lsub in range(4):
                    l0 = i * 512 + lsub * 128
                    xn = pb2.tile([128, D], BF16, tag="xn")
                    nc.sync.dma_start(out=xn, in_=x_d.ap()[l0:l0 + 128, :])
                    for dgrp in range(4):
                        ps = psS.tile([128, 512], BF16, tag="stream")
                        for k in range(4):
                            dch = dgrp * 4 + k
                            nc.tensor.transpose(
                                ps[:, k * 128:(k + 1) * 128],
                                xn[:, dch * 128:(dch + 1) * 128], ident)
                        nc.vector.tensor_copy(
                            out=xT[:, dgrp * 4:dgrp * 4 + 4,
                                   lsub * 128:(lsub + 1) * 128],
                            in_=ps.rearrange("p (a b) -> p a b", a=4))

                # q/k projection + rope (c = 0..3 q heads, c = 4 is k)
                for c in range(5):
                    wq_t = pb2.tile([128, NDCH, 128], BF16, tag="wqk")
                    nc.sync.dma_start(
                        out=wq_t,
                        in_=wqk_d.ap()[:, c * 128:(c + 1) * 128]
                            .rearrange("(t p) c -> p t c", p=128))
                    ps = psA.tile([128, 512], F32, tag="acc")
                    for t in range(NDCH):
                        nc.tensor.matmul(ps, lhsT=wq_t[:, t, :], rhs=xT[:, t, :],
                                         start=(t == 0), stop=(t == NDCH - 1))
                    isl = slice(i * 512, (i + 1) * 512)
                    dest = qT[:, c, isl] if c < NHQ else kT[:, isl]
                    cs = cosT[:, isl]
                    sn = sinT[:, isl]
                    tmp = pb2.tile([128, 512], F32, tag="rope")
                    nc.vector.tensor_mul(tmp[0:64, :], ps[64:128, :], sn[0:64, :])
                    nc.vector.tensor_mul(tmp[64:128, :], ps[0:64, :], sn[64:128, :])
                    tmp2 = pb2.tile([128, 512], F32, tag="rope2")
                    nc.vector.tensor_mul(tmp2, ps, cs)
                    nc.vector.tensor_sub(dest[0:64, :], tmp2[0:64, :], tmp[0:64, :])
                    nc.vector.tensor_add(dest[64:128, :], tmp2[64:128, :],
                                         tmp[64:128, :])

                # v projection for these 4 key chunks
                for lsub in range(4):
                    ps = psA.tile([128, 512], F32, tag="acc")
                    for t in range(NDCH):
                        nc.tensor.matmul(
                            ps, lhsT=xT[:, t, lsub * 128:(lsub + 1) * 128],
                            rhs=wv_sb[:, t, :],
                            start=(t == 0), stop=(t == NDCH - 1))
                    nc.scalar.copy(out=v_sb[:, i * 4 + lsub, :], in_=ps)

        # ---------------- Phase C+D: attention + fused out-projection -------
        with tc.tile_pool(name="pc1", bufs=1) as pc1, \
             tc.tile_pool(name="pc2", bufs=2) as pc2:
            for i in range(NI):
                isl = slice(i * 512, (i + 1) * 512)
                ctxTs = []
                for h in range(NHQ):
                    expS = pc2.tile([128, NJ, 512], BF16, tag="expS")
                    for j in range(NJ):
                        ps = psS.tile([128, 512], F32, tag="stream")
                        nc.tensor.matmul(ps, lhsT=kT[:, j * 128:(j + 1) * 128],
                                         rhs=qT[:, h, isl])
                        nc.scalar.activation(out=expS[:, j, :], in_=ps, func=EXP)
                    # row sums via ones-matmul (sum over keys = partition dim)
                    pso = psA.tile([1, 512], F32, tag="acc")
                    for j in range(NJ):
                        nc.tensor.matmul(pso, lhsT=ones[:, 0:1], rhs=expS[:, j, :],
                                         start=(j == 0), stop=(j == NJ - 1))
                    recip = pc1.tile([1, 512], F32, tag="recip")
                    nc.vector.reciprocal(recip, pso)
                    rb = pc2.tile([128, 512], F32, tag="rb")
                    nc.gpsimd.partition_broadcast(rb, recip)
                    ctxT = pc1.tile([128, 4, 512], BF16, tag=f"ctx{h}")
                    for ec in range(4):
                        ps = psA.tile([128, 512], F32, tag="acc")
                        for j in range(NJ):
                            nc.tensor.matmul(
                                ps, lhsT=v_sb[:, j, ec * 128:(ec + 1) * 128],
                                rhs=expS[:, j, :],
                                start=(j == 0), stop=(j == NJ - 1))
                        nc.vector.tensor_mul(ctxT[:, ec, :], ps, rb)
                    ctxTs.append(ctxT)

                # fused out-projection for this query chunk
                for dm in range(4):
                    wo_t = pc2.tile([128, NDCH, 512], BF16, tag="wo")
                    nc.sync.dma_start(
                        out=wo_t,
                        in_=wo_d.ap()[:, dm * 512:(dm + 1) * 512]
                            .rearrange("(t p) c -> p t c", p=128))
                    for lsub in range(4):
                        ps = psA.tile([128, 512], F32, tag="acc")
                        for h in range(NHQ):
                            for ec in range(4):
                                t = h * 4 + ec
                                nc.tensor.matmul(
                                    ps,
                                    lhsT=ctxTs[h][:, ec,
                                                  lsub * 128:(lsub + 1) * 128],
                                    rhs=wo_t[:, t, :],
                                    start=(t == 0), stop=(t == 15))
                        ost = pc2.tile([128, 512], F32, tag="ost")
                        nc.scalar.copy(out=ost, in_=ps)
                        l0 = i * 512 + lsub * 128
                        nc.sync.dma_start(
                            out=out_d.ap()[l0:l0 + 128, dm * 512:(dm + 1) * 512],
                            in_=ost)

    nc.compile()
    _NC_CACHE["nc"] = nc
    return nc


def make_core_inputs(x, W_attn, W_out):
    """Split full inputs into 8 per-core input maps (core = b*4 + g)."""
    Q_DIM = 2048
    K_DIM = 512
    scale = np.float32(1.0 / math.sqrt(DK))
    bf = ml_dtypes.bfloat16

    # rope tables, mirroring the fp32 reference computation
    inv_freq = (np.float32(1.0) /
                (np.float32(10000.0) **
                 (np.arange(0, DK, 2, dtype=np.float32) / np.float32(DK))))
    freqs = np.arange(L, dtype=np.float32)[:, None] * inv_freq[None, :]  # [L,64]
    ang = np.concatenate([freqs, freqs], axis=-1)  # [L, 128]
    cosT = np.ascontiguousarray(np.cos(ang).T.astype(np.float32))  # [128, L]
    sinT = np.ascontiguousarray(np.sin(ang).T.astype(np.float32))

    in_maps = []
    for core in range(8):
        b, g = divmod(core, 4)
        wq = (W_attn[:, 512 * g:512 * (g + 1)] * scale)
        wk = W_attn[:, Q_DIM + 128 * g:Q_DIM + 128 * (g + 1)]
        wqk = np.ascontiguousarray(
            np.concatenate([wq, wk], axis=1)).astype(bf)
        wv = np.ascontiguousarray(W_attn[:, Q_DIM + K_DIM + 512 * g:
                                         Q_DIM + K_DIM + 512 * (g + 1)]).astype(bf)
        wo = np.ascontiguousarray(W_out[2048 * g:2048 * (g + 1), :]).astype(bf)
        in_maps.append({
            "x": np.ascontiguousarray(x[b]).astype(bf),
            "wqk": wqk,
            "wv": wv,
            "wo": wo,
            "cost": cosT,
            "sint": sinT,
        })
    return in_maps


def kernel(x, W_attn, W_out, b_out, _trace=False, _trace_cores=None):
    x = np.asarray(x)
    W_attn = np.asarray(W_attn)
    W_out = np.asarray(W_out)
    b_out = np.asarray(b_out)
    nc = build_nc()
    in_maps = make_core_inputs(x, W_attn, W_out)
    res = run_bass_kernel_spmd(
        nc, in_maps, core_ids=list(range(8)),
        trace=_trace, trace_cores=_trace_cores)
    parts = [res.results[c]["out"] for c in range(8)]
    out = np.empty((2, L, D), dtype=np.float32)
    for b in range(2):
        acc = parts[4 * b].astype(np.float32)
        for g in range(1, 4):
            acc = acc + parts[4 * b + g]
        out[b] = acc + b_out[None, :].astype(np.float32)
    if _trace:
        kernel._last_results = res
    return out


# revision 7
# speedup vs baseline: 3.7330x; 1.0580x over previous
"""GQA kernel for trn2, 8 cores: DP over batch (2) x TP over kv-head groups (4).

Each core computes, for its (batch b, kv-group g):
  - qkv projection for its 4 q-heads + 1 kv-head (q pre-scaled by 1/sqrt(dk))
  - RoPE on q/k
  - full (non-causal) attention for the 4 q-heads vs its kv-head
  - partial out-projection with its 2048 rows of W_out
Host sums the 4 per-group partials per batch and adds bias.

Matmul operands are bf16 (PE runs fp32 at 1/4 rate; bf16 is full rate).
Accumulation and softmax statistics stay fp32.

Self-contained: hardcodes all shapes. kernel(**inputs) -> np.ndarray.
"""

import math
from contextlib import ExitStack

import numpy as np
import ml_dtypes

import concourse.bass as bass
import concourse.bacc as bacc
import concourse.tile as tile
import concourse.mybir as mybir
from concourse.bass_utils import run_bass_kernel_spmd
from concourse.masks import make_identity

F32 = mybir.dt.float32
BF16 = mybir.dt.bfloat16
L = 2048          # sequence length
D = 2048          # d_model
DK = 128          # head dim (q/k)
DV = 512          # head dim (v)
NHQ = 4           # q heads per core
CQK = NHQ * DK + DK   # 640 qk projection cols per core
NI = 4            # query chunks of 512
NJ = 16           # key chunks of 128
NDCH = 16         # d_model chunks of 128

_NC_CACHE = {}


def build_nc():
    if "nc" in _NC_CACHE:
        return _NC_CACHE["nc"]
    nc = bacc.Bacc("TRN2", target_bir_lowering=False, debug=False)

    x_d = nc.dram_tensor("x", [L, D], BF16, kind="ExternalInput")
    wqk_d = nc.dram_tensor("wqk", [D, CQK], BF16, kind="ExternalInput")
    wv_d = nc.dram_tensor("wv", [D, DV], BF16, kind="ExternalInput")
    wo_d = nc.dram_tensor("wo", [NHQ * DV, D], BF16, kind="ExternalInput")
    cos_d = nc.dram_tensor("cost", [DK, L], F32, kind="ExternalInput")
    sin_d = nc.dram_tensor("sint", [DK, L], F32, kind="ExternalInput")
    out_d = nc.dram_tensor("out", [L, D], F32, kind="ExternalOutput")

    EXP = mybir.ActivationFunctionType.Exp

    with ExitStack() as ctx:
        tc = ctx.enter_context(tile.TileContext(nc))
        # pools
        persist = ctx.enter_context(tc.tile_pool(name="persist", bufs=1))
        psS = ctx.enter_context(tc.tile_pool(name="psS", bufs=6, space="PSUM"))
        psA = ctx.enter_context(tc.tile_pool(name="psA", bufs=2, space="PSUM"))

        ident = persist.tile([128, 128], BF16)
        make_identity(nc, ident)
        ones = persist.tile([128, 1], BF16)
        nc.vector.memset(ones, 1.0)

        qT = persist.tile([128, NHQ, L], BF16)      # [dk, h, pos]
        kT = persist.tile([128, L], BF16)           # [dk, pos]
        v_sb = persist.tile([128, NJ, DV], BF16)    # [key_in_chunk, key_chunk, e]

        # ---------------- Phase B: x^T, qkv projection, rope ----------------
        with tc.tile_pool(name="pb1", bufs=1) as pb1, \
             tc.tile_pool(name="pb2", bufs=2) as pb2:
            cosT = pb1.tile([128, L], F32)
            sinT = pb1.tile([128, L], F32)
            nc.gpsimd.dma_start(out=cosT, in_=cos_d.ap())
            nc.gpsimd.dma_start(out=sinT, in_=sin_d.ap())
            wv_sb = pb1.tile([128, NDCH, DV], BF16)
            nc.gpsimd.dma_start(
                out=wv_sb, in_=wv_d.ap().rearrange("(t p) c -> p t c", p=128))

            for i in range(NI):
                xT = pb1.tile([128, NDCH, 512], BF16, tag="xT")
                # transpose x rows for this 512-query chunk
                for lsub in range(4):
                    l0 = i * 512 + lsub * 128
                    xn = pb2.tile([128, D], BF16, tag="xn")
                    nc.sync.dma_start(out=xn, in_=x_d.ap()[l0:l0 + 128, :])
                    for dgrp in range(4):
                        ps = psS.tile([128, 512], BF16, tag="stream")
                        for k in range(4):
                            dch = dgrp * 4 + k
                            nc.tensor.transpose(
                                ps[:, k * 128:(k + 1) * 128],
                                xn[:, dch * 128:(dch + 1) * 128], ident)
                        nc.vector.tensor_copy(
                            out=xT[:, dgrp * 4:dgrp * 4 + 4,
                                   lsub * 128:(lsub + 1) * 128],
                            in_=ps.rearrange("p (a b) -> p a b", a=4))

                # q/k projection + rope (c = 0..3 q heads, c = 4 is k)
                for c in range(5):
                    wq_t = pb2.tile([128, NDCH, 128], BF16, tag="wqk")
                    nc.sync.dma_start(
                        out=wq_t,
                        in_=wqk_d.ap()[:, c * 128:(c + 1) * 128]
                            .rearrange("(t p) c -> p t c", p=128))
                    ps = psA.tile([128, 512], F32, tag="acc")
                    for t in range(NDCH):
                        nc.tensor.matmul(ps, lhsT=wq_t[:, t, :], rhs=xT[:, t, :],
                                         start=(t == 0), stop=(t == NDCH - 1))
                    isl = slice(i * 512, (i + 1) * 512)
                    dest = qT[:, c, isl] if c < NHQ else kT[:, isl]
                    cs = cosT[:, isl]
                    sn = sinT[:, isl]
                    tmp = pb2.tile([128, 512], F32, tag="rope")
                    nc.vector.tensor_mul(tmp[0:64, :], ps[64:128, :], sn[0:64, :])
                    nc.vector.tensor_mul(tmp[64:128, :], ps[0:64, :], sn[64:128, :])
                    tmp2 = pb2.tile([128, 512], F32, tag="rope2")
                    nc.vector.tensor_mul(tmp2, ps, cs)
                    nc.vector.tensor_sub(dest[0:64, :], tmp2[0:64, :], tmp[0:64, :])
                    nc.vector.tensor_add(dest[64:128, :], tmp2[64:128, :],
                                         tmp[64:128, :])

                # v projection for these 4 key chunks
                for lsub in range(4):
                    ps = psA.tile([128, 512], F32, tag="acc")
                    for t in range(NDCH):
                        nc.tensor.matmul(
                            ps, lhsT=xT[:, t, lsub * 128:(lsub + 1) * 128],
                            rhs=wv_sb[:, t, :],
                            start=(t == 0), stop=(t == NDCH - 1))
                    nc.scalar.copy(out=v_sb[:, i * 4 + lsub, :], in_=ps)

        # ---------------- Phase C+D: attention + fused out-projection -------
        # software-pipelined: S/exp of pair k+1 is emitted before ones/PV of
        # pair k so ACT exp latency hides under PE's PV matmuls.
        with tc.tile_pool(name="pc1", bufs=1) as pc1, \
             tc.tile_pool(name="pc2", bufs=2) as pc2:
            ctxTs = {}

            def emit_s_exp(i, h):
                isl = slice(i * 512, (i + 1) * 512)
                expS = pc2.tile([128, NJ, 512], BF16, tag="expS")
                for j in range(NJ):
                    ps = psS.tile([128, 512], F32, tag="stream")
                    nc.tensor.matmul(ps, lhsT=kT[:, j * 128:(j + 1) * 128],
                                     rhs=qT[:, h, isl])
                    nc.scalar.activation(out=expS[:, j, :], in_=ps, func=EXP)
                return expS

            def emit_pv(i, h, expS):
                pso = psA.tile([1, 512], F32, tag="acc")
                for j in range(NJ):
                    nc.tensor.matmul(pso, lhsT=ones[:, 0:1], rhs=expS[:, j, :],
                                     start=(j == 0), stop=(j == NJ - 1))
                recip = pc1.tile([1, 512], F32, tag="recip")
                nc.vector.reciprocal(recip, pso)
                rb = pc2.tile([128, 512], F32, tag="rb")
                nc.gpsimd.partition_broadcast(rb, recip)
                ctxT = pc1.tile([128, 4, 512], BF16, tag=f"ctx{h}")
                for ec in range(4):
                    ps = psA.tile([128, 512], F32, tag="acc")
                    for j in range(NJ):
                        nc.tensor.matmul(
                            ps, lhsT=v_sb[:, j, ec * 128:(ec + 1) * 128],
                            rhs=expS[:, j, :],
                            start=(j == 0), stop=(j == NJ - 1))
                    nc.vector.tensor_mul(ctxT[:, ec, :], ps, rb)
                ctxTs[h] = ctxT

            def emit_outproj(i):
                for dm in range(4):
                    wo_t = pc2.tile([128, NDCH, 512], BF16, tag="wo")
                    nc.sync.dma_start(
                        out=wo_t,
                        in_=wo_d.ap()[:, dm * 512:(dm + 1) * 512]
                            .rearrange("(t p) c -> p t c", p=128))
                    for lsub in range(4):
                        ps = psA.tile([128, 512], F32, tag="acc")
                        for h in range(NHQ):
                            for ec in range(4):
                                t = h * 4 + ec
                                nc.tensor.matmul(
                                    ps,
                                    lhsT=ctxTs[h][:, ec,
                                                  lsub * 128:(lsub + 1) * 128],
                                    rhs=wo_t[:, t, :],
                                    start=(t == 0), stop=(t == 15))
                        ost = pc2.tile([128, 512], F32, tag="ost")
                        nc.scalar.copy(out=ost, in_=ps)
                        l0 = i * 512 + lsub * 128
                        nc.sync.dma_start(
                            out=out_d.ap()[l0:l0 + 128,
                                           dm * 512:(dm + 1) * 512],
                            in_=ost)

            pairs = [(i, h) for i in range(NI) for h in range(NHQ)]
            prev = None
            for (i, h) in pairs:
                cur = (i, h, emit_s_exp(i, h))
                if prev is not None:
                    pi, ph, pexp = prev
                    emit_pv(pi, ph, pexp)
                    if ph == NHQ - 1:
                        emit_outproj(pi)
                prev = cur
            pi, ph, pexp = prev
            emit_pv(pi, ph, pexp)
            emit_outproj(pi)

    nc.compile()
    _NC_CACHE["nc"] = nc
    return nc


def make_core_inputs(x, W_attn, W_out):
    """Split full inputs into 8 per-core input maps (core = b*4 + g)."""
    Q_DIM = 2048
    K_DIM = 512
    scale = np.float32(1.0 / math.sqrt(DK))
    bf = ml_dtypes.bfloat16

    # rope tables, mirroring the fp32 reference computation
    inv_freq = (np.float32(1.0) /
                (np.float32(10000.0) **
                 (np.arange(0, DK, 2, dtype=np.float32) / np.float32(DK))))
    freqs = np.arange(L, dtype=np.float32)[:, None] * inv_freq[None, :]  # [L,64]
    ang = np.concatenate([freqs, freqs], axis=-1)  # [L, 128]
    cosT = np.ascontiguousarray(np.cos(ang).T.astype(np.float32))  # [128, L]
    sinT = np.ascontiguousarray(np.sin(ang).T.astype(np.float32))

    in_maps = []
    for core in range(8):
        b, g = divmod(core, 4)
        wq = (W_attn[:, 512 * g:512 * (g + 1)] * scale)
        wk = W_attn[:, Q_DIM + 128 * g:Q_DIM + 128 * (g + 1)]
        wqk = np.ascontiguousarray(
            np.concatenate([wq, wk], axis=1)).astype(bf)
        wv = np.ascontiguousarray(W_attn[:, Q_DIM + K_DIM + 512 * g:
                                         Q_DIM + K_DIM + 512 * (g + 1)]).astype(bf)
        wo = np.ascontiguousarray(W_out[2048 * g:2048 * (g + 1), :]).astype(bf)
        in_maps.append({
            "x": np.ascontiguousarray(x[b]).astype(bf),
            "wqk": wqk,
            "wv": wv,
            "wo": wo,
            "cost": cosT,
            "sint": sinT,
        })
    return in_maps


def kernel(x, W_attn, W_out, b_out, _trace=False, _trace_cores=None):
    x = np.asarray(x)
    W_attn = np.asarray(W_attn)
    W_out = np.asarray(W_out)
    b_out = np.asarray(b_out)
    nc = build_nc()
    in_maps = make_core_inputs(x, W_attn, W_out)
    res = run_bass_kernel_spmd(
        nc, in_maps, core_ids=list(range(8)),
        trace=_trace, trace_cores=_trace_cores)
    parts = [res.results[c]["out"] for c in range(8)]
    out = np.empty((2, L, D), dtype=np.float32)
    for b in range(2):
        acc = parts[4 * b].astype(np.float32)
        for g in range(1, 4):
            acc = acc + parts[4 * b + g]
        out[b] = acc + b_out[None, :].astype(np.float32)
    if _trace:
        kernel._last_results = res
    return out
